# revision 1
# baseline (speedup 1.0000x reference)
"""LeNet-style CNN (conv5x5+avgpool2+sigmoid x2, then 3 FC layers) on 8 trn2
NeuronCores, pure data parallel over the batch.

Key ideas:
- conv(5x5, stride 1) followed by 2x2 avg-pool is algebraically a single
  6x6 stride-2 convolution (pooling is linear) -> each conv layer is one
  strided conv: 28x28 -> 12x12 (10ch), 12x12 -> 4x4 (20ch).
- Activations live in SBUF as [feature partitions, (spatial-major, batch)]
  free layout. The strided convs then become accumulating matmuls with
  Toeplitz-structured weights reading *strided views* of the previous
  layer - no im2col data movement on device at all.
- x is pre-transposed on the host to [(row-parity, width)=56 partitions,
  (row-half, batch)] so the device DMA is fully contiguous and the row
  stride-2 of layer 1 turns into partition-parity selection.
- bf16 matmul inputs (fp32 PSUM accumulation) for the conv/FC1/FC2 stages;
  FC3 (84->10, final linear) runs in true fp32. Bias+sigmoid fuse into one
  ScalarE activation per 1024-wide PSUM tile.
- Input DMAs split across both HWDGE rings (sync + scalar) with weights on
  the gpsimd SWDGE ring so transfers overlap instead of serializing.
"""

import numpy as np
import ml_dtypes
import concourse.bacc as bacc
import concourse.mybir as mybir
import concourse.tile as tile
from concourse.vector_clock import ScopedClock
from concourse.bass_utils import run_bass_kernel_spmd

F32 = mybir.dt.float32
F32R = mybir.dt.float32r
BF16 = mybir.dt.bfloat16
SIG = mybir.ActivationFunctionType.Sigmoid

N_CORES = 8
B_FULL = 8192
NB = B_FULL // N_CORES  # 1024 images per core
HB = 512  # matmul moving-dim tile (one PSUM bank of fp32)


class SlimTailTileContext(tile.TileContext):
    """Tile's standard teardown emits drain + all-engine barrier + semaphore
    clears + another barrier (~10us on HW). This NEFF executes exactly once
    per load, so the semaphore-reset choreography is dead weight: keep the
    data-completeness drain, do the allocator bookkeeping host-side only."""

    def _drain_and_barrier(self, tick_clock, wait_clock):
        drain_inst = self.nc.sync.drain()
        wait_clock.add_sem_waits(
            drain_inst.ins, ScopedClock({None: tick_clock.global_clock})
        )
        popped = self.nc._tile_sem_poison_stack.pop()
        assert popped is self._sem_poison
        sems = list(self.sems.allocated().values())
        sem_nums = [sm.num for sm in sems]
        self.nc._state.prepend_free_semaphores(sem_nums)
        for poison_set in self.nc._tile_sem_poison_stack:
            poison_set.update(sem_nums)


def _fuse_pool(W):
    """conv(W, stride 1) + 2x2 mean-pool == conv(Wf, stride 2), Wf 6x6."""
    O, C, _, _ = W.shape
    Wf = np.zeros((O, C, 6, 6), np.float32)
    for u in (0, 1):
        for v in (0, 1):
            Wf[:, :, u : u + 5, v : v + 5] += W
    return Wf * 0.25


def _host_weights(W1, b1, W2, b2, L1, Lb1, L2, Lb2, L3, Lb3):
    W1f = _fuse_pool(W1)  # [10,1,6,6]
    W2f = _fuse_pool(W2)  # [20,10,6,6]

    # Layer 1 Toeplitz: T1[k][(par,w), (pj,o)] = W1f[o,0,2k+par,w-2pj]
    T1 = np.zeros((3, 56, 120), np.float32)
    for k in range(3):
        for par in range(2):
            e = 2 * k + par
            for pj in range(12):
                for f in range(6):
                    w = 2 * pj + f
                    T1[k, par * 28 + w, pj * 10 : pj * 10 + 10] = W1f[:, 0, e, f]

    # Layer 2 Toeplitz: T2[par*3+k][(pj,c), (qj,oc)] = W2f[oc,c,2k+par,pj-2qj]
    T2 = np.zeros((6, 120, 80), np.float32)
    for par in range(2):
        for k in range(3):
            e = 2 * k + par
            for qj in range(4):
                for f in range(6):
                    pj = 2 * qj + f
                    for c in range(10):
                        T2[par * 3 + k, pj * 10 + c, qj * 20 : qj * 20 + 20] = W2f[
                            :, c, e, f
                        ]

    # FC1 permuted for the [(qj,oc) partitions, (qi,b) free] input layout:
    # 4 accumulation steps over qi.
    L1p = np.zeros((4, 80, 120), np.float32)
    for qi in range(4):
        for qj in range(4):
            for oc in range(20):
                L1p[qi, qj * 20 + oc] = L1[oc * 16 + qi * 4 + qj]

    bf = ml_dtypes.bfloat16
    bias1 = np.ascontiguousarray(
        np.tile(np.asarray(b1).reshape(10), 12).reshape(120, 1), dtype=np.float32
    )
    bias2 = np.ascontiguousarray(
        np.tile(np.asarray(b2).reshape(20), 4).reshape(80, 1), dtype=np.float32
    )
    lb1 = np.ascontiguousarray(np.asarray(Lb1).reshape(120, 1), dtype=np.float32)
    lb2 = np.ascontiguousarray(np.asarray(Lb2).reshape(84, 1), dtype=np.float32)
    # FC3 bias folded in via a constant-one activation row.
    L3b = np.ascontiguousarray(
        np.concatenate([np.asarray(L3), np.asarray(Lb3).reshape(1, 10)], axis=0),
        dtype=np.float32,
    )  # [85, 10]
    return {
        # packed so each weight group is one DMA
        "t1": np.ascontiguousarray(T1.transpose(1, 0, 2).reshape(56, 360), dtype=bf),
        "t2": np.ascontiguousarray(T2.transpose(1, 0, 2).reshape(120, 480), dtype=bf),
        "l1p": np.ascontiguousarray(L1p.transpose(1, 0, 2).reshape(80, 480), dtype=np.float32),
        "l2w": np.ascontiguousarray(L2, dtype=np.float32),  # [120,84]
        "l3b": L3b,
        "bias1": bias1,
        "bias2": bias2,
        "lb1": lb1,
        "lb2": lb2,
    }


def _build_nc():
    nc = bacc.Bacc()
    xp = nc.dram_tensor("xp", [56, 14 * NB], BF16, kind="ExternalInput")
    t1 = nc.dram_tensor("t1", [56, 360], BF16, kind="ExternalInput")
    t2 = nc.dram_tensor("t2", [120, 480], BF16, kind="ExternalInput")
    l1p = nc.dram_tensor("l1p", [80, 480], F32R, kind="ExternalInput")
    l2w = nc.dram_tensor("l2w", [120, 84], F32R, kind="ExternalInput")
    l3b = nc.dram_tensor("l3b", [85, 10], F32R, kind="ExternalInput")
    bias1 = nc.dram_tensor("bias1", [120, 1], F32, kind="ExternalInput")
    bias2 = nc.dram_tensor("bias2", [80, 1], F32, kind="ExternalInput")
    lb1 = nc.dram_tensor("lb1", [120, 1], F32, kind="ExternalInput")
    lb2 = nc.dram_tensor("lb2", [84, 1], F32, kind="ExternalInput")
    y = nc.dram_tensor("y", [128, 80], F32, kind="ExternalOutput")

    with SlimTailTileContext(nc) as tc:
        with (
            tc.tile_pool(name="w", bufs=1) as wp,
            tc.tile_pool(name="act", bufs=1) as ap,
            tc.tile_pool(name="ps", bufs=3, space="PSUM") as psp,
            tc.tile_pool(name="psy", bufs=2, space="PSUM") as psyp,
        ):
            # --- warm-up scaffolding: the PE HAM clock-gate starts at
            # 1.2 GHz and only reaches 2.4 GHz after ~3.4us of sustained
            # matmul activity; a dependency-free dummy matmul burst during
            # the input DMA phase warms it before the real matmuls start.
            # A dummy sigmoid also forces the ACT table load off the
            # critical path.
            warm = wp.tile([128, 640], BF16, tag="warm")
            nc.vector.memset(warm[:, :], 0.0)
            warmf = wp.tile([128, 16], F32, tag="warmf")
            nc.vector.memset(warmf[:, :], 0.0)
            nc.scalar.activation(warmf[:, 8:16], warmf[:, 0:8], SIG)
            for i in range(14):
                wps = psyp.tile([128, HB], F32, tag="psy")
                nc.tensor.matmul(
                    wps[:], warm[:, :128], warm[:, 128:640], start=True, stop=True
                )

            # --- weights + input x: spread across both HWDGE rings (sync +
            # scalar); t1 first so layer 1 can start as soon as x chunk 0
            # lands. gpsimd only memsets (SWDGE drains are expensive).
            xs = ap.tile([56, 14 * NB], BF16, tag="xp")
            t1s = wp.tile([56, 360], BF16, tag="t1")
            t2s = wp.tile([120, 480], BF16, tag="t2")
            l1s = wp.tile([80, 480], F32R, tag="l1p")
            l2s = wp.tile([120, 84], F32R, tag="l2w")
            l3s = wp.tile([85, 10], F32R, tag="l3b")
            b1s = wp.tile([120, 1], F32, tag="bias1")
            b2s = wp.tile([80, 1], F32, tag="bias2")
            lb1s = wp.tile([120, 1], F32, tag="lb1")
            lb2s = wp.tile([84, 1], F32, tag="lb2")

            def xchunk(c):
                sl = slice(c * 2 * NB, (c + 1) * 2 * NB)
                nc.sync.dma_start(xs[:, sl], xp[:, sl])

            # everything on the sync HWDGE ring, ordered by first use;
            # splitting across the scalar ring measured SLOWER (ring
            # contention delays completions and blocks ACT behind DMAs)
            nc.sync.dma_start(t1s[:], t1[:])
            xchunk(0)
            xchunk(1)
            nc.sync.dma_start(b1s[:], bias1[:])
            xchunk(2)
            xchunk(3)
            nc.sync.dma_start(t2s[:], t2[:])
            nc.sync.dma_start(b2s[:], bias2[:])
            xchunk(4)
            xchunk(5)
            xchunk(6)
            nc.sync.dma_start(l1s[:], l1p[:])
            nc.sync.dma_start(lb1s[:], lb1[:])
            nc.sync.dma_start(l2s[:], l2w[:])
            nc.sync.dma_start(lb2s[:], lb2[:])
            nc.sync.dma_start(l3s[:], l3b[:])

            # --- activations ---
            # h1: [(pj,o)=120, free = par*6*NB + pih*NB + b], pi = 2*pih+par
            h1 = ap.tile([120, 12 * NB], BF16, tag="h1")
            # h2: [(qj,oc)=80, free = qi*NB + b]
            h2 = ap.tile([80, 4 * NB], F32R, tag="h2")
            h3 = ap.tile([120, NB], F32R, tag="h3")
            h4 = ap.tile([85, NB], F32R, tag="h4")  # row 84 == 1.0 (FC3 bias)
            ys = ap.tile([128, 80], F32, tag="ys")

            # Row 84 must be 1.0 (FC3 bias row); FC2's activation later
            # overwrites rows 0..83, so filling the whole tile is fine and
            # keeps the memset base-partition at 0 (gpsimd alignment rule).
            nc.gpsimd.memset(h4[:, :].bitcast(F32), 1.0)

            # --- layer 1: per pi, 2 halves x 3 accumulating matmuls, 1 ACT
            for pi in range(12):
                hoff = ((pi % 2) * 6 + pi // 2) * NB
                ps = psp.tile([120, NB], F32, tag="ps")
                for h in range(2):
                    b0 = h * HB
                    for k in range(3):
                        rhs = xs[:, (pi + k) * NB + b0 : (pi + k) * NB + b0 + HB]
                        nc.tensor.matmul(
                            ps[:, b0 : b0 + HB],
                            t1s[:, k * 120 : (k + 1) * 120],
                            rhs,
                            start=(k == 0),
                            stop=(k == 2),
                        )
                nc.scalar.activation(h1[:, hoff : hoff + NB], ps[:], SIG, bias=b1s[:])

            # --- layer 2: per qi, 2 halves x 6 accumulating matmuls, 1 ACT
            for qi in range(4):
                ps = psp.tile([80, NB], F32, tag="ps")
                for h in range(2):
                    b0 = h * HB
                    n = 0
                    for par in range(2):
                        for k in range(3):
                            off = (par * 6 + (qi + k)) * NB + b0
                            i6 = par * 3 + k
                            nc.tensor.matmul(
                                ps[:, b0 : b0 + HB],
                                t2s[:, i6 * 80 : (i6 + 1) * 80],
                                h1[:, off : off + HB],
                                start=(n == 0),
                                stop=(n == 5),
                            )
                            n += 1
                    nc.scalar.activation(
                        h2[:, qi * NB + b0 : qi * NB + b0 + HB],
                        ps[:, b0 : b0 + HB],
                        SIG,
                        bias=b2s[:],
                    )

            # --- FC1 (320->120): 4 accumulating matmuls over qi per half ---
            ps1 = psp.tile([120, NB], F32, tag="ps")
            for h in range(2):
                b0 = h * HB
                for qi in range(4):
                    nc.tensor.matmul(
                        ps1[:, b0 : b0 + HB],
                        l1s[:, qi * 120 : (qi + 1) * 120],
                        h2[:, qi * NB + b0 : qi * NB + b0 + HB],
                        start=(qi == 0),
                        stop=(qi == 3),
                    )
                nc.scalar.activation(
                    h3[:, b0 : b0 + HB], ps1[:, b0 : b0 + HB], SIG, bias=lb1s[:]
                )

            # --- FC2 (120->84) ---
            ps2 = psp.tile([84, NB], F32, tag="ps")
            for h in range(2):
                b0 = h * HB
                nc.tensor.matmul(
                    ps2[:, b0 : b0 + HB],
                    l2s[:],
                    h3[:, b0 : b0 + HB],
                    start=True,
                    stop=True,
                )
                nc.scalar.activation(
                    h4[0:84, b0 : b0 + HB], ps2[:, b0 : b0 + HB], SIG, bias=lb2s[:]
                )

            # --- FC3 (84->10, bias via ones row): activations stationary
            for t4 in range(8):
                ps = psyp.tile([128, 10], F32, tag="psy")
                nc.tensor.matmul(
                    ps[:],
                    h4[:, t4 * 128 : (t4 + 1) * 128],
                    l3s[:],
                    start=True,
                    stop=True,
                )
                nc.vector.tensor_copy(ys[:, t4 * 10 : (t4 + 1) * 10], ps[:])

            # --- output: partition-contiguous [128, 80]; host unpermutes
            nc.sync.dma_start(y[:, :], ys[:, :])
    nc.compile()
    return nc


_NC_CACHE = None


def _get_nc():
    global _NC_CACHE
    if _NC_CACHE is None:
        _NC_CACHE = _build_nc()
    return _NC_CACHE


def _make_in_maps(x, W1, b1, W2, b2, L1, Lb1, L2, Lb2, L3, Lb3):
    wmap = _host_weights(W1, b1, W2, b2, L1, Lb1, L2, Lb2, L3, Lb3)
    x = np.asarray(x, dtype=np.float32)
    in_maps = []
    for c in range(N_CORES):
        xc = x[c * NB : (c + 1) * NB, 0]  # [NB, 28, 28]
        xpc = np.ascontiguousarray(
            xc.reshape(NB, 14, 2, 28).transpose(2, 3, 1, 0).reshape(56, 14 * NB),
            dtype=ml_dtypes.bfloat16,
        )
        m = {"xp": xpc}
        m.update(wmap)
        in_maps.append(m)
    return in_maps


def _run(trace=False, **inputs):
    global _NC_CACHE
    nc = _get_nc()
    in_maps = _make_in_maps(**inputs)
    res = run_bass_kernel_spmd(nc, in_maps, list(range(N_CORES)), trace=trace)
    # the slim teardown leaves semaphores dirty; force a fresh NEFF if
    # kernel() is ever called again in this process
    _NC_CACHE = None
    outs = []
    for i in range(N_CORES):
        yc = res.results[i]["y"]  # [128, 80] = [p, (t, n)]
        outs.append(yc.reshape(128, 8, 10).transpose(1, 0, 2).reshape(NB, 10))
    out = np.ascontiguousarray(np.concatenate(outs, axis=0))
    return out, res


def kernel(**inputs):
    out, _ = _run(trace=False, **inputs)
    return out



# revision 23
# speedup vs baseline: 1.2595x; 1.2595x over previous
"""LeNet-style CNN (conv5x5+avgpool2+sigmoid x2, then 3 FC layers) on 8 trn2
NeuronCores, pure data parallel over the batch.

v2 key ideas (on top of the fused-conv v1):
- conv+pool fused to a 6x6 stride-2 conv (pooling is linear), expressed as
  accumulating matmuls with Toeplitz weights over strided SBUF views.
- Layer-1 K=120 merging: x is laid out host-side as
  [(row mod 4, width) -> partitions 0-55 (m=0,1) and 64-119 (m=2,3), zeros in
  56-63], so the three K=56 contributions per output row merge into TWO
  K=120 full-array accumulating matmuls (adjacent kernel-row pairs read the
  same 4-row group; the unused half of each stationary is zero): 72 serial
  matmul slots become 48. (True A/B row-strip pairing dies on HW: an
  accumulation group must keep one tile_position across its matmuls.)
- h1 is stored [120 parts = (pj,o), block pi, batch], so one merged
  bias+sigmoid ACT per L1 pair covers [120, 2, 1024] (fewer ScalarE
  fixed overheads; ScalarE is the co-bottleneck).
- L1 pairs and L2 qi-tiles are interleaved (p0 p1 p2 q0 p3 q1 p4 q2 p5 q3)
  to balance PE (L2-heavy) vs ScalarE (L1-heavy) load.
- FC3 runs as a moving-batch matmul ([85,10] stationary, batch moving) so
  the output is [10, 1024]: 2 matmuls + 2 vector copies instead of 8
  stationary-activation tiles, with per-half output DMA.
- Weights packed into 4 DMAs total; x in 4 group-chunk DMAs, all on the
  sync HWDGE ring, ordered by first use.
"""

import numpy as np
import ml_dtypes
import concourse.bacc as bacc
import concourse.mybir as mybir
import concourse.tile as tile
from concourse.vector_clock import ScopedClock
from concourse.bass_utils import run_bass_kernel_spmd

F32 = mybir.dt.float32
F32R = mybir.dt.float32r
BF16 = mybir.dt.bfloat16
FP8 = mybir.dt.float8e4
SIG = mybir.ActivationFunctionType.Sigmoid

N_CORES = 8
B_FULL = 8192
NB = B_FULL // N_CORES  # 1024 images per core
HB = 512  # matmul moving-dim tile (one PSUM bank of fp32)
N_WARM = 7


class SlimTailTileContext(tile.TileContext):
    """Tile's standard teardown emits drain + all-engine barrier + semaphore
    clears + another barrier (~10us on HW). This NEFF executes exactly once
    per load, so the semaphore-reset choreography is dead weight: keep the
    data-completeness drain, do the allocator bookkeeping host-side only."""

    def _drain_and_barrier(self, tick_clock, wait_clock):
        drain_inst = self.nc.sync.drain()
        wait_clock.add_sem_waits(
            drain_inst.ins, ScopedClock({None: tick_clock.global_clock})
        )
        popped = self.nc._tile_sem_poison_stack.pop()
        assert popped is self._sem_poison
        sems = list(self.sems.allocated().values())
        sem_nums = [sm.num for sm in sems]
        self.nc._state.prepend_free_semaphores(sem_nums)
        for poison_set in self.nc._tile_sem_poison_stack:
            poison_set.update(sem_nums)


def _fuse_pool(W):
    """conv(W, stride 1) + 2x2 mean-pool == conv(Wf, stride 2), Wf 6x6."""
    O, C, _, _ = W.shape
    Wf = np.zeros((O, C, 6, 6), np.float32)
    for u in (0, 1):
        for v in (0, 1):
            Wf[:, :, u : u + 5, v : v + 5] += W
    return Wf * 0.25


def _host_weights(W1, b1, W2, b2, L1, Lb1, L2, Lb2, L3, Lb3):
    W1f = _fuse_pool(np.asarray(W1, np.float32))  # [10,1,6,6]
    W2f = _fuse_pool(np.asarray(W2, np.float32))  # [20,10,6,6]

    # Layer 1 Toeplitz: S_k[(m',w), (pj,o)] = W1f[o, 0, 2k+m', w-2pj],
    # merged into 4 zero-padded [120,120] stationaries (partition halves
    # 0-55 / 64-119 are the two kernel-row-pair positions of a 4-row group):
    # blk0 = [S_0; 0; S_1] (pi even, first group), blk1 = [S_2; 0; 0]
    # blk2 = [0; 0; S_0] (pi odd, first group),   blk3 = [S_1; 0; S_2]
    S = np.zeros((3, 56, 120), np.float32)
    for k in range(3):
        for mp in range(2):
            e = 2 * k + mp
            for pj in range(12):
                for f in range(6):
                    w = 2 * pj + f
                    S[k, mp * 28 + w, pj * 10 : pj * 10 + 10] = W1f[:, 0, e, f]
    t1 = np.zeros((120, 4, 128), np.float32)
    t1[0:56, 0, 0:120] = S[0]
    t1[64:120, 0, 0:120] = S[1]
    t1[0:56, 1, 0:120] = S[2]
    t1[64:120, 2, 0:120] = S[0]
    t1[0:56, 3, 0:120] = S[1]
    t1[64:120, 3, 0:120] = S[2]

    # Layer 2 Toeplitz: T2[par*3+k][(pj,c), (qj,oc)] = W2f[oc,c,2k+par,pj-2qj]
    t2 = np.zeros((120, 480), np.float32)
    for par in range(2):
        for k in range(3):
            e = 2 * k + par
            i6 = par * 3 + k
            for qj in range(4):
                for f in range(6):
                    pj = 2 * qj + f
                    for c in range(10):
                        t2[pj * 10 + c, i6 * 80 + qj * 20 : i6 * 80 + qj * 20 + 20] = (
                            W2f[:, c, e, f]
                        )

    # biases + final linear packed into one tiny [120, 14] fp32 DMA:
    # col 0: bias1 (120), col 1: bias2 (80), col 2: lb1 (120), col 3: lb2 (84),
    # cols 4:14: [L3; Lb3] (85 rows)
    bt = np.zeros((120, 14), np.float32)
    bt[:, 0] = np.tile(np.asarray(b1, np.float32).reshape(10), 12)
    bt[0:80, 1] = np.tile(np.asarray(b2, np.float32).reshape(20), 4)
    bt[:, 2] = np.asarray(Lb1, np.float32).reshape(120)
    bt[0:84, 3] = np.asarray(Lb2, np.float32).reshape(84)
    bt[0:84, 4:14] = np.asarray(L3, np.float32)
    bt[84, 4:14] = np.asarray(Lb3, np.float32).reshape(10)

    # FC1 permuted for the [(qj,oc) partitions, (qi,b) free] input layout,
    # packed with the FC2 matrix into one [120, 564] fp32 DMA.
    wfc = np.zeros((120, 564), np.float32)
    L1a = np.asarray(L1, np.float32)
    for qi in range(4):
        for qj in range(4):
            for oc in range(20):
                wfc[qj * 20 + oc, qi * 120 : (qi + 1) * 120] = L1a[
                    oc * 16 + qi * 4 + qj
                ]
    wfc[:, 480:564] = np.asarray(L2, np.float32)  # [120, 84]

    bf = ml_dtypes.bfloat16
    return {
        "t1": np.ascontiguousarray(t1, dtype=ml_dtypes.float8_e4m3),
        "t2": np.ascontiguousarray(t2, dtype=bf),
        "bt": np.ascontiguousarray(bt),
        "wfc": np.ascontiguousarray(wfc),
    }


def _build_nc():
    nc = bacc.Bacc()
    xp = nc.dram_tensor("xp", [120, 7, NB], FP8, kind="ExternalInput")
    t2 = nc.dram_tensor("t2", [120, 480], BF16, kind="ExternalInput")
    # F32R so the verifier accepts these as fp32r-matmul inputs (same bits)
    t1 = nc.dram_tensor("t1", [120, 4, 128], FP8, kind="ExternalInput")
    bt = nc.dram_tensor("bt", [120, 14], F32R, kind="ExternalInput")
    wfc = nc.dram_tensor("wfc", [120, 564], F32R, kind="ExternalInput")
    y = nc.dram_tensor("y", [10, NB], F32, kind="ExternalOutput")

    with SlimTailTileContext(nc) as tc:
        with (
            tc.tile_pool(name="w", bufs=1) as wp,
            tc.tile_pool(name="act", bufs=1) as ap,
            tc.tile_pool(name="ps", bufs=4, space="PSUM") as psp,
        ):
            # --- warm-up scaffolding: PE HAM clock-gate starts at 1.2 GHz and
            # reaches 2.4 GHz only after ~3.4us of sustained matmul activity;
            # dependency-free dummy matmuls during the input DMA phase warm it.
            # A dummy sigmoid also pulls the ACT table load off the critical
            # path.
            bts = wp.tile([120, 14], F32R, tag="bt")
            warm = wp.tile([128, 640], BF16, tag="warm")
            nc.vector.memset(warm[:, :], 0.0)
            warmf = wp.tile([128, 16], F32, tag="warmf")
            nc.vector.memset(warmf[:, :], 0.0)
            nc.scalar.activation(warmf[:, 8:16], warmf[:, 0:8], SIG)
            for _ in range(N_WARM):
                wps = psp.tile([128, HB], F32, tag="ps")
                nc.tensor.matmul(
                    wps[:], warm[:, :128], warm[:, 128:640], start=True, stop=True
                )

            # --- SBUF residents ---
            xs = ap.tile([120, 7, NB], FP8, tag="xp")
            t2s = wp.tile([120, 480], BF16, tag="t2")
            t1s = wp.tile([120, 4, 128], FP8, tag="t1")
            wfcs = wp.tile([120, 564], F32R, tag="wfc")

            b1s = bts[:, 0:1].bitcast(F32)
            b2s = bts[0:80, 1:2].bitcast(F32)
            lb1s = bts[:, 2:3].bitcast(F32)
            lb2s = bts[0:84, 3:4].bitcast(F32)
            l3s = bts[0:85, 4:14]
            l1s = wfcs[0:80, 0:480]
            l2s = wfcs[:, 480:564]

            h1 = ap.tile([120, 12 * NB], BF16, tag="h1")  # [.., block pi * NB + b]
            h2 = ap.tile([80, 4 * NB], F32R, tag="h2")  # [(qj,oc), qi*NB+b]
            h3 = ap.tile([120, NB], F32R, tag="h3")
            h4 = ap.tile([85, NB], F32R, tag="h4")  # row 84 == 1.0 (FC3 bias)
            ys = ap.tile([10, NB], F32, tag="ys")

            # Row 84 must be 1.0 (FC3 bias row); FC2's activation later
            # overwrites rows 0..83.
            nc.gpsimd.memset(h4[:, :].bitcast(F32), 1.0)

            # --- input DMAs, sync HWDGE ring, ordered by first use ---
            def xchunk(g0, g1):
                nc.sync.dma_start(xs[:, g0:g1, :], xp[:, g0:g1, :])

            nc.sync.dma_start(t1s[:], t1[:])
            xchunk(0, 2)
            xchunk(2, 4)
            nc.sync.dma_start(bts[:], bt[:])
            nc.sync.dma_start(t2s[:], t2[:])
            xchunk(4, 6)
            xchunk(6, 7)
            nc.sync.dma_start(wfcs[:], wfc[:])

            # --- layer 1, output row pi: two K=120 full-array accumulating
            # matmuls per (pi, half) over x groups g=pi//2 and pi//2+1;
            # stationaries zero-padded so each contributes only its valid
            # kernel-row pairs. (An accumulation group must keep ONE
            # tile_position across its matmuls on HW, so K=56 A/B row-strip
            # pairing is not an option.)
            # fp8 DoubleRow: both K=120 group-contributions ride one matmul
            # (virtual 256-row array; stationary [120, 2, 128] middle dim =
            # group step, moving [120, 2, 512] = x groups g, g+1).
            def l1_pi(pi):
                ps = psp.tile([128, 1024], F32, tag="ps")
                g = pi // 2
                p = pi % 2
                for h in range(2):
                    b0 = h * HB
                    nc.tensor.matmul(
                        ps[:, b0 : b0 + HB],
                        t1s[:, 2 * p : 2 * p + 2, :],
                        xs[:, g : g + 2, b0 : b0 + HB],
                        start=True,
                        stop=True,
                        perf_mode=mybir.MatmulPerfMode.DoubleRow,
                    )
                nc.scalar.activation(
                    h1[:, pi * NB : (pi + 1) * NB], ps[0:120, :], SIG, bias=b1s
                )

            # --- layer 2 output-row qi: 2 halves x 6 accumulating matmuls,
            # one merged ACT per qi.
            l2_ps = {}

            def l2_qi_mms(qi, ks):
                if qi not in l2_ps:
                    l2_ps[qi] = psp.tile([80, 1024], F32, tag="ps", name=f"psq{qi}")
                ps = l2_ps[qi]
                for h in range(2):
                    b0 = h * HB
                    for k in ks:
                        for par in range(2):
                            blk = 2 * (qi + k) + par
                            i6 = par * 3 + k
                            nc.tensor.matmul(
                                ps[:, h * HB : h * HB + HB],
                                t2s[:, i6 * 80 : (i6 + 1) * 80],
                                h1[:, blk * NB + b0 : blk * NB + b0 + HB],
                                start=(k == 0 and par == 0),
                                stop=(k == 2 and par == 1),
                            )

            def l2_qi(qi):
                ps = l2_ps[qi]
                if qi == 3:
                    for h in range(2):
                        b0 = h * HB
                        nc.scalar.activation(
                            h2[:, qi * NB + b0 : qi * NB + b0 + HB],
                            ps[:, b0 : b0 + HB],
                            SIG,
                            bias=b2s,
                        )
                else:
                    nc.scalar.activation(
                        h2[:, qi * NB : (qi + 1) * NB], ps[:, :], SIG, bias=b2s
                    )

            # interleave at k-granularity: q_i's step k needs p_(2i+2k+1);
            # emitting (k0,k1) after p_(2i+3) and (k2 + ACT) after p_(2i+5)
            # lets the PE fill h1-wait bubbles with ready work.
            l1_pi(0)
            l1_pi(1)
            l1_pi(2)
            l1_pi(3)
            l2_qi_mms(0, (0, 1))
            l1_pi(4)
            l1_pi(5)
            l2_qi_mms(0, (2,))
            l2_qi(0)
            l2_qi_mms(1, (0, 1))
            l1_pi(6)
            l1_pi(7)
            l2_qi_mms(1, (2,))
            l2_qi(1)
            l2_qi_mms(2, (0, 1))
            l1_pi(8)
            l1_pi(9)
            l2_qi_mms(2, (2,))
            l2_qi(2)
            l2_qi_mms(3, (0, 1))
            l1_pi(10)
            l1_pi(11)
            l2_qi_mms(3, (2,))
            l2_qi(3)

            # --- FC1/FC2/FC3 pipeline, per-half psum tiles so the two
            # batch halves overlap (a shared tile serializes h1 matmuls
            # behind h0's ACT read). FC3 is batch-moving ([85,10] stationary)
            # with per-half copy + output DMA so the tail drains early.
            ps1 = [psp.tile([120, HB], F32, tag="ps", name=f"ps1{h}") for h in range(2)]
            QB = 256
            ps2 = [psp.tile([84, QB], F32, tag="ps", name=f"ps2{q}") for q in range(4)]
            ps3 = [psp.tile([10, QB], F32, tag="ps", name=f"ps3{q}") for q in range(4)]
            for h in range(2):
                b0 = h * HB
                for qi in range(4):
                    nc.tensor.matmul(
                        ps1[h][:, :],
                        l1s[:, qi * 120 : (qi + 1) * 120],
                        h2[:, qi * NB + b0 : qi * NB + b0 + HB],
                        start=(qi == 0),
                        stop=(qi == 3),
                    )
                nc.scalar.activation(
                    h3[:, b0 : b0 + HB], ps1[h][:, :], SIG, bias=lb1s
                )
            for q in range(4):
                b0 = q * QB
                nc.tensor.matmul(
                    ps2[q][:, :], l2s, h3[:, b0 : b0 + QB], start=True, stop=True
                )
                nc.scalar.activation(
                    h4[0:84, b0 : b0 + QB], ps2[q][:, :], SIG, bias=lb2s
                )
            for q in range(4):
                b0 = q * QB
                nc.tensor.matmul(
                    ps3[q][:, :], l3s, h4[:, b0 : b0 + QB], start=True, stop=True
                )
                nc.vector.tensor_copy(ys[:, b0 : b0 + QB], ps3[q][:, :])
            nc.sync.dma_start(y[:, :], ys[:, :])
    nc.compile()
    return nc


_NC_CACHE = None


def _get_nc():
    global _NC_CACHE
    if _NC_CACHE is None:
        _NC_CACHE = _build_nc()
    return _NC_CACHE


def _make_in_maps(x, W1, b1, W2, b2, L1, Lb1, L2, Lb2, L3, Lb3):
    wmap = _host_weights(W1, b1, W2, b2, L1, Lb1, L2, Lb2, L3, Lb3)
    x = np.asarray(x, dtype=np.float32)
    bf = ml_dtypes.bfloat16
    in_maps = []
    for c in range(N_CORES):
        xc = x[c * NB : (c + 1) * NB, 0]  # [NB, 28, 28]
        # rows r = 4g + m; partitions: m in {0,1} -> 0:56, m in {2,3} -> 64:120
        v = xc.reshape(NB, 7, 4, 28).transpose(2, 3, 1, 0).reshape(112, 7, NB)
        xpc = np.zeros((120, 7, NB), dtype=ml_dtypes.float8_e4m3)
        xpc[0:56] = v[0:56]
        xpc[64:120] = v[56:112]
        m = {"xp": xpc}
        m.update(wmap)
        in_maps.append(m)
    return in_maps


def _run(trace=False, **inputs):
    global _NC_CACHE
    nc = _get_nc()
    in_maps = _make_in_maps(**inputs)
    res = run_bass_kernel_spmd(nc, in_maps, list(range(N_CORES)), trace=trace)
    # the slim teardown leaves semaphores dirty; force a fresh NEFF if
    # kernel() is ever called again in this process
    _NC_CACHE = None
    outs = []
    for i in range(N_CORES):
        yc = res.results[i]["y"]  # [10, NB]
        outs.append(yc.T)
    out = np.ascontiguousarray(np.concatenate(outs, axis=0))
    return out, res


def kernel(**inputs):
    out, _ = _run(trace=False, **inputs)
    return out


# revision 24
# speedup vs baseline: 1.3425x; 1.0659x over previous
"""LeNet-style CNN (conv5x5+avgpool2+sigmoid x2, then 3 FC layers) on 8 trn2
NeuronCores, pure data parallel over the batch.

v2 key ideas (on top of the fused-conv v1):
- conv+pool fused to a 6x6 stride-2 conv (pooling is linear), expressed as
  accumulating matmuls with Toeplitz weights over strided SBUF views.
- Layer-1 K=120 merging: x is laid out host-side as
  [(row mod 4, width) -> partitions 0-55 (m=0,1) and 64-119 (m=2,3), zeros in
  56-63], so the three K=56 contributions per output row merge into TWO
  K=120 full-array accumulating matmuls (adjacent kernel-row pairs read the
  same 4-row group; the unused half of each stationary is zero): 72 serial
  matmul slots become 48. (True A/B row-strip pairing dies on HW: an
  accumulation group must keep one tile_position across its matmuls.)
- h1 is stored [120 parts = (pj,o), block pi, batch], so one merged
  bias+sigmoid ACT per L1 pair covers [120, 2, 1024] (fewer ScalarE
  fixed overheads; ScalarE is the co-bottleneck).
- L1 pairs and L2 qi-tiles are interleaved (p0 p1 p2 q0 p3 q1 p4 q2 p5 q3)
  to balance PE (L2-heavy) vs ScalarE (L1-heavy) load.
- FC3 runs as a moving-batch matmul ([85,10] stationary, batch moving) so
  the output is [10, 1024]: 2 matmuls + 2 vector copies instead of 8
  stationary-activation tiles, with per-half output DMA.
- Weights packed into 4 DMAs total; x in 4 group-chunk DMAs, all on the
  sync HWDGE ring, ordered by first use.
"""

import numpy as np
import ml_dtypes
import concourse.bacc as bacc
import concourse.mybir as mybir
import concourse.tile as tile
from concourse.vector_clock import ScopedClock
from concourse.bass_utils import run_bass_kernel_spmd

F32 = mybir.dt.float32
F32R = mybir.dt.float32r
BF16 = mybir.dt.bfloat16
FP8 = mybir.dt.float8e4
SIG = mybir.ActivationFunctionType.Sigmoid

N_CORES = 8
B_FULL = 8192
NB = B_FULL // N_CORES  # 1024 images per core
HB = 512  # matmul moving-dim tile (one PSUM bank of fp32)
N_WARM = 7


class SlimTailTileContext(tile.TileContext):
    """Tile's standard teardown emits drain + all-engine barrier + semaphore
    clears + another barrier (~10us on HW). This NEFF executes exactly once
    per load, so the semaphore-reset choreography is dead weight: keep the
    data-completeness drain, do the allocator bookkeeping host-side only."""

    def _drain_and_barrier(self, tick_clock, wait_clock):
        drain_inst = self.nc.sync.drain()
        wait_clock.add_sem_waits(
            drain_inst.ins, ScopedClock({None: tick_clock.global_clock})
        )
        popped = self.nc._tile_sem_poison_stack.pop()
        assert popped is self._sem_poison
        sems = list(self.sems.allocated().values())
        sem_nums = [sm.num for sm in sems]
        self.nc._state.prepend_free_semaphores(sem_nums)
        for poison_set in self.nc._tile_sem_poison_stack:
            poison_set.update(sem_nums)


def _fuse_pool(W):
    """conv(W, stride 1) + 2x2 mean-pool == conv(Wf, stride 2), Wf 6x6."""
    O, C, _, _ = W.shape
    Wf = np.zeros((O, C, 6, 6), np.float32)
    for u in (0, 1):
        for v in (0, 1):
            Wf[:, :, u : u + 5, v : v + 5] += W
    return Wf * 0.25


def _host_weights(W1, b1, W2, b2, L1, Lb1, L2, Lb2, L3, Lb3):
    W1f = _fuse_pool(np.asarray(W1, np.float32))  # [10,1,6,6]
    W2f = _fuse_pool(np.asarray(W2, np.float32))  # [20,10,6,6]

    # Layer 1 Toeplitz: S_k[(m',w), (pj,o)] = W1f[o, 0, 2k+m', w-2pj],
    # merged into 4 zero-padded [120,120] stationaries (partition halves
    # 0-55 / 64-119 are the two kernel-row-pair positions of a 4-row group):
    # blk0 = [S_0; 0; S_1] (pi even, first group), blk1 = [S_2; 0; 0]
    # blk2 = [0; 0; S_0] (pi odd, first group),   blk3 = [S_1; 0; S_2]
    S = np.zeros((3, 56, 120), np.float32)
    for k in range(3):
        for mp in range(2):
            e = 2 * k + mp
            for pj in range(12):
                for f in range(6):
                    w = 2 * pj + f
                    S[k, mp * 28 + w, pj * 10 : pj * 10 + 10] = W1f[:, 0, e, f]
    t1 = np.zeros((120, 4, 128), np.float32)
    t1[0:56, 0, 0:120] = S[0]
    t1[64:120, 0, 0:120] = S[1]
    t1[0:56, 1, 0:120] = S[2]
    t1[64:120, 2, 0:120] = S[0]
    t1[0:56, 3, 0:120] = S[1]
    t1[64:120, 3, 0:120] = S[2]

    # Layer 2 Toeplitz, fp8 DoubleRow layout: block j2 = 2k+par so moving
    # h1 block pairs (2(qi+k), 2(qi+k)+1) pair with stationary blocks
    # (2k, 2k+1); free dim padded 80 -> 96 for alignment.
    t2 = np.zeros((120, 6, 96), np.float32)
    for par in range(2):
        for k in range(3):
            e = 2 * k + par
            j2 = 2 * k + par
            for qj in range(4):
                for f in range(6):
                    pj = 2 * qj + f
                    for c in range(10):
                        t2[pj * 10 + c, j2, qj * 20 : qj * 20 + 20] = W2f[:, c, e, f]

    # biases + final linear packed into one tiny [120, 14] fp32 DMA:
    # col 0: bias1 (120), col 1: bias2 (80), col 2: lb1 (120), col 3: lb2 (84),
    # cols 4:14: [L3; Lb3] (85 rows)
    bt = np.zeros((120, 14), np.float32)
    bt[:, 0] = np.tile(np.asarray(b1, np.float32).reshape(10), 12)
    bt[0:80, 1] = np.tile(np.asarray(b2, np.float32).reshape(20), 4)
    bt[:, 2] = np.asarray(Lb1, np.float32).reshape(120)
    bt[0:84, 3] = np.asarray(Lb2, np.float32).reshape(84)
    bt[0:84, 4:14] = np.asarray(L3, np.float32)
    bt[84, 4:14] = np.asarray(Lb3, np.float32).reshape(10)

    # FC1 permuted for the [(qj,oc) partitions, (qi,b) free] input layout,
    # packed with the FC2 matrix into one [120, 564] fp32 DMA.
    wfc = np.zeros((120, 564), np.float32)
    L1a = np.asarray(L1, np.float32)
    for qi in range(4):
        for qj in range(4):
            for oc in range(20):
                wfc[qj * 20 + oc, qi * 120 : (qi + 1) * 120] = L1a[
                    oc * 16 + qi * 4 + qj
                ]
    wfc[:, 480:564] = np.asarray(L2, np.float32)  # [120, 84]

    bf = ml_dtypes.bfloat16
    return {
        "t1": np.ascontiguousarray(t1, dtype=ml_dtypes.float8_e4m3),
        "t2": np.ascontiguousarray(t2, dtype=ml_dtypes.float8_e4m3),
        "bt": np.ascontiguousarray(bt),
        "wfc": np.ascontiguousarray(wfc),
    }


def _build_nc():
    nc = bacc.Bacc()
    xp = nc.dram_tensor("xp", [120, 7, NB], FP8, kind="ExternalInput")
    t2 = nc.dram_tensor("t2", [120, 6, 96], FP8, kind="ExternalInput")
    # F32R so the verifier accepts these as fp32r-matmul inputs (same bits)
    t1 = nc.dram_tensor("t1", [120, 4, 128], FP8, kind="ExternalInput")
    bt = nc.dram_tensor("bt", [120, 14], F32R, kind="ExternalInput")
    wfc = nc.dram_tensor("wfc", [120, 564], F32R, kind="ExternalInput")
    y = nc.dram_tensor("y", [10, NB], F32, kind="ExternalOutput")

    with SlimTailTileContext(nc) as tc:
        with (
            tc.tile_pool(name="w", bufs=1) as wp,
            tc.tile_pool(name="act", bufs=1) as ap,
            tc.tile_pool(name="ps", bufs=4, space="PSUM") as psp,
        ):
            # --- warm-up scaffolding: PE HAM clock-gate starts at 1.2 GHz and
            # reaches 2.4 GHz only after ~3.4us of sustained matmul activity;
            # dependency-free dummy matmuls during the input DMA phase warm it.
            # A dummy sigmoid also pulls the ACT table load off the critical
            # path.
            bts = wp.tile([120, 14], F32R, tag="bt")
            warm = wp.tile([128, 640], BF16, tag="warm")
            nc.vector.memset(warm[:, :], 0.0)
            warmf = wp.tile([128, 16], F32, tag="warmf")
            nc.vector.memset(warmf[:, :], 0.0)
            nc.scalar.activation(warmf[:, 8:16], warmf[:, 0:8], SIG)
            for _ in range(N_WARM):
                wps = psp.tile([128, HB], F32, tag="ps")
                nc.tensor.matmul(
                    wps[:], warm[:, :128], warm[:, 128:640], start=True, stop=True
                )

            # --- SBUF residents ---
            xs = ap.tile([120, 7, NB], FP8, tag="xp")
            t2s = wp.tile([120, 6, 96], FP8, tag="t2")
            t1s = wp.tile([120, 4, 128], FP8, tag="t1")
            wfcs = wp.tile([120, 564], F32R, tag="wfc")

            b1s = bts[:, 0:1].bitcast(F32)
            b2s = bts[0:80, 1:2].bitcast(F32)
            lb1s = bts[:, 2:3].bitcast(F32)
            lb2s = bts[0:84, 3:4].bitcast(F32)
            l3s = bts[0:85, 4:14]
            l1s = wfcs[0:80, 0:480]
            l2s = wfcs[:, 480:564]

            h1 = ap.tile([120, 12, NB], FP8, tag="h1")  # [.., block pi * NB + b]
            h2 = ap.tile([80, 4 * NB], F32R, tag="h2")  # [(qj,oc), qi*NB+b]
            h3 = ap.tile([120, NB], F32R, tag="h3")
            h4 = ap.tile([85, NB], F32R, tag="h4")  # row 84 == 1.0 (FC3 bias)
            ys = ap.tile([10, NB], F32, tag="ys")

            # Row 84 must be 1.0 (FC3 bias row); FC2's activation later
            # overwrites rows 0..83.
            nc.gpsimd.memset(h4[:, :].bitcast(F32), 1.0)

            # --- input DMAs, sync HWDGE ring, ordered by first use ---
            def xchunk(g0, g1):
                nc.sync.dma_start(xs[:, g0:g1, :], xp[:, g0:g1, :])

            nc.sync.dma_start(t1s[:], t1[:])
            xchunk(0, 2)
            xchunk(2, 4)
            nc.sync.dma_start(bts[:], bt[:])
            nc.sync.dma_start(t2s[:], t2[:])
            xchunk(4, 6)
            xchunk(6, 7)
            nc.sync.dma_start(wfcs[:], wfc[:])

            # --- layer 1, output row pi: two K=120 full-array accumulating
            # matmuls per (pi, half) over x groups g=pi//2 and pi//2+1;
            # stationaries zero-padded so each contributes only its valid
            # kernel-row pairs. (An accumulation group must keep ONE
            # tile_position across its matmuls on HW, so K=56 A/B row-strip
            # pairing is not an option.)
            # fp8 DoubleRow: both K=120 group-contributions ride one matmul
            # (virtual 256-row array; stationary [120, 2, 128] middle dim =
            # group step, moving [120, 2, 512] = x groups g, g+1).
            def l1_pi(pi):
                ps = psp.tile([128, 1024], F32, tag="ps")
                g = pi // 2
                p = pi % 2
                for h in range(2):
                    b0 = h * HB
                    nc.tensor.matmul(
                        ps[:, b0 : b0 + HB],
                        t1s[:, 2 * p : 2 * p + 2, :],
                        xs[:, g : g + 2, b0 : b0 + HB],
                        start=True,
                        stop=True,
                        perf_mode=mybir.MatmulPerfMode.DoubleRow,
                    )
                nc.scalar.activation(
                    h1[:, pi, :], ps[0:120, :], SIG, bias=b1s
                )

            # --- layer 2 output-row qi: 2 halves x 6 accumulating matmuls,
            # one merged ACT per qi.
            l2_ps = {}

            def l2_qi_mms(qi, ks):
                if qi not in l2_ps:
                    l2_ps[qi] = psp.tile([96, 1024], F32, tag="ps", name=f"psq{qi}")
                ps = l2_ps[qi]
                for h in range(2):
                    b0 = h * HB
                    for k in ks:
                        nc.tensor.matmul(
                            ps[:, h * HB : h * HB + HB],
                            t2s[:, 2 * k : 2 * k + 2, :],
                            h1[:, 2 * (qi + k) : 2 * (qi + k) + 2, b0 : b0 + HB],
                            start=(k == 0),
                            stop=(k == 2),
                            perf_mode=mybir.MatmulPerfMode.DoubleRow,
                        )

            def l2_qi(qi):
                ps = l2_ps[qi]
                if qi == 3:
                    for h in range(2):
                        b0 = h * HB
                        nc.scalar.activation(
                            h2[:, qi * NB + b0 : qi * NB + b0 + HB],
                            ps[0:80, b0 : b0 + HB],
                            SIG,
                            bias=b2s,
                        )
                else:
                    nc.scalar.activation(
                        h2[:, qi * NB : (qi + 1) * NB], ps[0:80, :], SIG, bias=b2s
                    )

            # interleave at k-granularity: q_i's step k needs p_(2i+2k+1);
            # emitting (k0,k1) after p_(2i+3) and (k2 + ACT) after p_(2i+5)
            # lets the PE fill h1-wait bubbles with ready work.
            l1_pi(0)
            l1_pi(1)
            l1_pi(2)
            l1_pi(3)
            l2_qi_mms(0, (0, 1))
            l1_pi(4)
            l1_pi(5)
            l2_qi_mms(0, (2,))
            l2_qi(0)
            l2_qi_mms(1, (0, 1))
            l1_pi(6)
            l1_pi(7)
            l2_qi_mms(1, (2,))
            l2_qi(1)
            l2_qi_mms(2, (0, 1))
            l1_pi(8)
            l1_pi(9)
            l2_qi_mms(2, (2,))
            l2_qi(2)
            l2_qi_mms(3, (0, 1))
            l1_pi(10)
            l1_pi(11)
            l2_qi_mms(3, (2,))
            l2_qi(3)

            # --- FC1/FC2/FC3 pipeline, per-half psum tiles so the two
            # batch halves overlap (a shared tile serializes h1 matmuls
            # behind h0's ACT read). FC3 is batch-moving ([85,10] stationary)
            # with per-half copy + output DMA so the tail drains early.
            ps1 = [psp.tile([120, HB], F32, tag="ps", name=f"ps1{h}") for h in range(2)]
            QB = 256
            ps2 = [psp.tile([84, QB], F32, tag="ps", name=f"ps2{q}") for q in range(4)]
            ps3 = [psp.tile([10, QB], F32, tag="ps", name=f"ps3{q}") for q in range(4)]
            for h in range(2):
                b0 = h * HB
                for qi in range(4):
                    nc.tensor.matmul(
                        ps1[h][:, :],
                        l1s[:, qi * 120 : (qi + 1) * 120],
                        h2[:, qi * NB + b0 : qi * NB + b0 + HB],
                        start=(qi == 0),
                        stop=(qi == 3),
                    )
                nc.scalar.activation(
                    h3[:, b0 : b0 + HB], ps1[h][:, :], SIG, bias=lb1s
                )
            for q in range(4):
                b0 = q * QB
                nc.tensor.matmul(
                    ps2[q][:, :], l2s, h3[:, b0 : b0 + QB], start=True, stop=True
                )
                nc.scalar.activation(
                    h4[0:84, b0 : b0 + QB], ps2[q][:, :], SIG, bias=lb2s
                )
            for q in range(4):
                b0 = q * QB
                nc.tensor.matmul(
                    ps3[q][:, :], l3s, h4[:, b0 : b0 + QB], start=True, stop=True
                )
                nc.vector.tensor_copy(ys[:, b0 : b0 + QB], ps3[q][:, :])
            nc.sync.dma_start(y[:, :], ys[:, :])
    nc.compile()
    return nc


_NC_CACHE = None


def _get_nc():
    global _NC_CACHE
    if _NC_CACHE is None:
        _NC_CACHE = _build_nc()
    return _NC_CACHE


def _make_in_maps(x, W1, b1, W2, b2, L1, Lb1, L2, Lb2, L3, Lb3):
    wmap = _host_weights(W1, b1, W2, b2, L1, Lb1, L2, Lb2, L3, Lb3)
    x = np.asarray(x, dtype=np.float32)
    bf = ml_dtypes.bfloat16
    in_maps = []
    for c in range(N_CORES):
        xc = x[c * NB : (c + 1) * NB, 0]  # [NB, 28, 28]
        # rows r = 4g + m; partitions: m in {0,1} -> 0:56, m in {2,3} -> 64:120
        v = xc.reshape(NB, 7, 4, 28).transpose(2, 3, 1, 0).reshape(112, 7, NB)
        xpc = np.zeros((120, 7, NB), dtype=ml_dtypes.float8_e4m3)
        xpc[0:56] = v[0:56]
        xpc[64:120] = v[56:112]
        m = {"xp": xpc}
        m.update(wmap)
        in_maps.append(m)
    return in_maps


def _run(trace=False, **inputs):
    global _NC_CACHE
    nc = _get_nc()
    in_maps = _make_in_maps(**inputs)
    res = run_bass_kernel_spmd(nc, in_maps, list(range(N_CORES)), trace=trace)
    # the slim teardown leaves semaphores dirty; force a fresh NEFF if
    # kernel() is ever called again in this process
    _NC_CACHE = None
    outs = []
    for i in range(N_CORES):
        yc = res.results[i]["y"]  # [10, NB]
        outs.append(yc.T)
    out = np.ascontiguousarray(np.concatenate(outs, axis=0))
    return out, res


def kernel(**inputs):
    out, _ = _run(trace=False, **inputs)
    return out
